# revision 40
# baseline (speedup 1.0000x reference)
"""Multi-head attention kernel for Trainium2, SPMD across 8 NeuronCores.

Problem: b=4, n=2048, h=16 heads, d=64/head, per-head projections with
shared [64,64] weights, pair mask, softmax, out = attn @ v.

Sharding: (batch, head) units are fully independent -> shard heads across
cores (2 heads/core, all 4 batches). No collectives.

Per-core device algorithm (per batch b, head-pair):
  qT_pair[128, n] = blockdiag(Wq', Wq')^T-style matmul vs host-transposed
                    X^T (both heads at once, scale 1/8 folded into Wq).
  kT_pair likewise; v projected to natural [n, 128] layout tiles with an
  extra ones column per head (vhat [128j, 65]).
  S^T tiles [128j, 512i] per head via row-packed matmuls (two K=64 heads
  occupy PE rows 0-63 / 64-127 concurrently).
  E = exp(S^T + bias_j), bias_j = -30*(1-mask_j) folds the column mask;
  no row max needed (scores are O(0.1); constant offsets cancel in
  normalization). exp is split ~2:1 between ScalarE (table exp, per-
  partition bias) and VectorE (Schraudolph bit-trick exp via tensor_scalar
  with int16 output bitcast to bf16) to balance the two engines.
  outT[65, i] += vhat[j]^T-stationary matmul over E; column 64 (ones)
  accumulates the softmax denominator for free.
  DMA-transpose outT (fp16) back to natural [i, 65] tiles, DVE normalize,
  and blend fully-masked rows with vbar = mean of all v rows (reference
  semantics: masked queries attend uniformly to ALL keys).

Measured (wall-clock slope over an on-device For_i loop; NTFF profiling is
unavailable in this dev container): ~250-270 us per full forward across all
8 cores, down from 457 us for the first correct version via, in order:
psO accumulator double-buffering (-80), half-tile S-PSUM pipeline (-11),
a software-pipeline skew of the out-matmuls past the exp stage on the
in-order PE (skew=2, -50), merging the per-head vhat tiles (-45), and
replacing the 16-matmul vbar accumulation chain with a DVE row-reduce +
single projection + PE-transpose + K=1 broadcast matmul (-48; the old
chain also pinned a PSUM accumulator slot at every batch start).
All deltas from paired interleaved A/B. Relative error vs the f32
reference: 0.0059 (gate 2e-2).
"""

import numpy as np
import ml_dtypes

NUM_HEADS = 16
D = 64
B = 4
N = 2048
NCORES = 8
P = 128
MASK_BIAS = -30.0

BF16 = ml_dtypes.bfloat16
F16 = np.float16

# Schraudolph-style exp for bf16 bit space: exp(x) ~ bitcast_bf16(int16(x*A + B))
# A = 2^7/ln2; B centers the piecewise-linear error (C calibrated so the mean
# ratio vs exp() is 1.0 for |x|<~0.2, +0.25 splits round-vs-trunc ambiguity).
SCHRAUD_A = 128.0 / np.log(2.0)
SCHRAUD_B = 127.0 * 128 - 1.2 + 0.25

_GRAPH_CACHE = {}


def _build_graph(b_count=B, n=N, reps=1, probe=None, probe_amt=4, skew=2, vmerge=True, ebufs=6, novbar=True):
    import concourse.bass as bass
    import concourse.mybir as mybir
    import concourse.tile as tile
    from concourse import bacc

    from concourse.alu_op_type import AluOpType

    nt = n // P            # number of 128-wide j tiles
    niq = max(n // 512, 1)  # i-quarters
    iqw = min(n, 512)       # i-chunk width
    dt = mybir.dt
    AF = mybir.ActivationFunctionType

    nc = bacc.Bacc()

    xqT_d = nc.declare_dram_parameter("xqT", [b_count, P, n], dt.bfloat16, isOutput=False)
    xkT_d = nc.declare_dram_parameter("xkT", [b_count, P, n], dt.bfloat16, isOutput=False)
    xvT_d = nc.declare_dram_parameter("xvT", [b_count, P, n], dt.bfloat16, isOutput=False)
    wq_d = nc.declare_dram_parameter("wq", [P, P], dt.bfloat16, isOutput=False)
    wk_d = nc.declare_dram_parameter("wk", [P, P], dt.bfloat16, isOutput=False)
    wv_d = nc.declare_dram_parameter("wv", [P, P], dt.bfloat16, isOutput=False)
    mask_d = nc.declare_dram_parameter("maskt", [b_count, P, nt], dt.uint8, isOutput=False)
    gbias_d = nc.declare_dram_parameter("gbias", [b_count, P, nt], dt.float32, isOutput=False)
    gbias2_d = nc.declare_dram_parameter("gbias2", [b_count, P, nt], dt.float32, isOutput=False)
    ident_d = nc.declare_dram_parameter("ident", [P, P], dt.bfloat16, isOutput=False) if novbar else None
    out_d = nc.declare_dram_parameter("out", [b_count, n, P], dt.float32, isOutput=True)
    probe_d = nc.declare_dram_parameter("probe_out", [1, 4], dt.float32, isOutput=True) if probe else None

    with tile.TileContext(nc) as tc:
        with (
            tc.tile_pool(name="const", bufs=1) as cpool,
            tc.tile_pool(name="xin", bufs=2) as xpool,
            tc.tile_pool(name="qk", bufs=4) as qkpool,
            tc.tile_pool(name="vhat", bufs=4) as vpool,
            tc.tile_pool(name="e", bufs=ebufs) as epool,
            tc.tile_pool(name="outT", bufs=4) as opool,
            tc.tile_pool(name="nat", bufs=4) as npool,
            tc.tile_pool(name="fin", bufs=4) as fpool,
            tc.tile_pool(name="vb", bufs=2) as vbpool,
            tc.tile_pool(name="psA", bufs=(3 if probe else 4), space="PSUM") as psA,
            tc.tile_pool(name="psO", bufs=2, space="PSUM") as psO,
        ):
            # constants
            wq_t = cpool.tile([P, P], dt.bfloat16)
            wk_t = cpool.tile([P, P], dt.bfloat16)
            wv_t = cpool.tile([P, P], dt.bfloat16)
            ones_t = cpool.tile([P, 1], dt.bfloat16, tag="ones")
            ones_bc = cpool.tile([1, P], dt.bfloat16, tag="ones_bc")
            nc.sync.dma_start(wq_t[:], wq_d[:])
            nc.sync.dma_start(wk_t[:], wk_d[:])
            nc.sync.dma_start(wv_t[:], wv_d[:])
            nc.gpsimd.memset(ones_t[:], 1.0 / n)
            nc.gpsimd.memset(ones_bc[:], 1.0)
            if novbar:
                ident_t = cpool.tile([P, P], dt.bfloat16, tag="ident")
                nc.sync.dma_start(ident_t[:], ident_d[:])

            mask_t = [
                cpool.tile([P, nt], dt.uint8, tag=f"mask{b}", name=f"mask_t{b}")
                for b in range(b_count)
            ]
            gbias_t = [
                cpool.tile([P, nt], dt.float32, tag=f"gbias{b}", name=f"gbias_t{b}")
                for b in range(b_count)
            ]
            gbias2_t = [
                cpool.tile([P, nt], dt.float32, tag=f"gbias2{b}", name=f"gbias2_t{b}")
                for b in range(b_count)
            ]
            for b in range(b_count):
                nc.sync.dma_start(mask_t[b][:], mask_d[b])
                nc.sync.dma_start(gbias_t[b][:], gbias_d[b])
                nc.sync.dma_start(gbias2_t[b][:], gbias2_d[b])

            if probe:
                pr_in = cpool.tile([P, 2048], dt.float32, tag="pr_in")
                pr_inb = cpool.tile([P, 512], dt.bfloat16, tag="pr_inb")
                pr_sc = cpool.tile([P, 2048], dt.float32, tag="pr_sc")
                pr_w = cpool.tile([P, P], dt.bfloat16, tag="pr_w")
                nc.gpsimd.memset(pr_in[:], 0.125)
                nc.gpsimd.memset(pr_inb[:], 0.125)
                nc.gpsimd.memset(pr_w[:], 0.5)
                nc.gpsimd.memset(pr_sc[:], 0.0)
                prps_cm = tc.tile_pool(name="prps", bufs=1, space="PSUM")
                prps_pool = prps_cm.__enter__()

            def emit_probe():
                if not probe:
                    return
                if probe == "act":
                    for _ in range(probe_amt):
                        nc.scalar.activation(pr_sc[:], pr_in[:], AF.Exp, bias=0.0)
                elif probe == "dve":
                    for _ in range(probe_amt):
                        nc.vector.tensor_copy(pr_sc[:], pr_in[:])
                elif probe == "pe":
                    prps = prps_pool.tile([P, 512], dt.float32, tag="prps", name="prps")
                    for _ in range(probe_amt):
                        nc.tensor.matmul(prps[:], pr_w[:], pr_inb[:])
                elif probe == "pe128":
                    prps = prps_pool.tile([P, 512], dt.float32, tag="prps", name="prps")
                    for _ in range(probe_amt):
                        nc.tensor.matmul(prps[:, 0:128], pr_w[:], pr_inb[:, 0:128])
                elif probe == "peldw1":
                    prps = prps_pool.tile([P, 512], dt.float32, tag="prps", name="prps")
                    for _ in range(probe_amt):
                        nc.tensor.matmul(prps[0:1, :], pr_w[:, 0:1], pr_inb[:])
                elif probe == "sp":
                    for _ in range(probe_amt):
                        nc.sync.dma_start(pr_sc[:, 0:1024], xqT_d[0][:, 0:1024])

            import contextlib

            rep_ctx = tc.For_i(0, reps, 1) if reps > 1 else contextlib.nullcontext()
            with rep_ctx:
              for b in range(b_count):
                # ---- stage inputs ----
                xq_s = xpool.tile([P, n], dt.bfloat16, tag="xq")
                xk_s = xpool.tile([P, n], dt.bfloat16, tag="xk")
                xv_s = xpool.tile([P, n], dt.bfloat16, tag="xv")
                nc.sync.dma_start(xq_s[:], xqT_d[b])
                nc.sync.dma_start(xk_s[:], xkT_d[b])
                nc.sync.dma_start(xv_s[:], xvT_d[b])

                # ---- q/k projections (both heads stacked on partitions) ----
                qT = qkpool.tile([P, n], dt.bfloat16, tag="qk")
                kT = qkpool.tile([P, n], dt.bfloat16, tag="qk")
                for dst, w_t, src in ((qT, wq_t, xq_s), (kT, wk_t, xk_s)):
                    for t in range(0, n, 512):
                        w = min(512, n - t)
                        pp = psA.tile([P, w], dt.float32, tag="ps")
                        nc.tensor.matmul(pp[:], w_t[:], src[:, t : t + w])
                        nc.vector.tensor_copy(dst[:, t : t + w], pp[:])

                # ---- v projection -> vhat [128, nt, 130]: per head 64 v cols +
                # a ones column (64/129) that accumulates softmax denominators
                if vmerge:
                    vhat = vpool.tile([P, nt, 130], dt.bfloat16, tag="vhat")
                    vh0 = vhat[:, :, 0:65]
                    vh1 = vhat[:, :, 65:130]
                    nc.gpsimd.memset(vhat[:, :, 64:65], 1.0)
                    nc.gpsimd.memset(vhat[:, :, 129:130], 1.0)
                else:
                    vhat0 = vpool.tile([P, nt, 65], dt.bfloat16, tag="vhat")
                    vhat1 = vpool.tile([P, nt, 65], dt.bfloat16, tag="vhat")
                    vh0, vh1 = vhat0[:, :, :], vhat1[:, :, :]
                    nc.gpsimd.memset(vhat0[:, :, 64:65], 1.0)
                    nc.gpsimd.memset(vhat1[:, :, 64:65], 1.0)
                for t in range(nt // 4):
                    pv = psA.tile([P, 512], dt.float32, tag="ps")
                    for c in range(4):
                        j = 4 * t + c
                        nc.tensor.matmul(
                            pv[:, c * 128 : (c + 1) * 128],
                            xv_s[:, j * 128 : (j + 1) * 128],
                            wv_t[:],
                        )
                    pv3 = pv[:].rearrange("p (c m) -> p c m", c=4)
                    nc.vector.tensor_copy(vh0[:, 4 * t : 4 * t + 4, 0:64], pv3[:, :, 0:64])
                    nc.vector.tensor_copy(vh1[:, 4 * t : 4 * t + 4, 0:64], pv3[:, :, 64:128])

                # ---- vbar = mean over all n rows of v (both heads) ----
                vbar_bc = vbpool.tile([P, P], dt.float16, tag="vbar_bc")
                if novbar:
                    # sum Xv over n on DVE, project once, PE-transpose to a
                    # free-dim row, then K=1 broadcast matmul
                    xvbar = vbpool.tile([P, 1], dt.float32, tag="xvbar")
                    nc.vector.reduce_sum(xvbar[:], xv_s[:], axis=bass.mybir.AxisListType.X)
                    xvbar_b = vbpool.tile([P, 1], dt.bfloat16, tag="xvbar_b")
                    nc.vector.tensor_copy(xvbar_b[:], xvbar[:])
                    pvb = psO.tile([P, 1], dt.float32, tag="o")
                    nc.tensor.matmul(pvb[:], wv_t[:], xvbar_b[:])
                    vbt_sb = vbpool.tile([P, 1], dt.bfloat16, tag="vbt_sb")
                    nc.vector.tensor_copy(vbt_sb[:], pvb[:])
                    pvrow = psO.tile([1, P], dt.bfloat16, tag="o")
                    nc.tensor.transpose(pvrow[:], vbt_sb[:], ident_t[:])
                    vrow_sb = vbpool.tile([1, P], dt.bfloat16, tag="vrow_sb")
                    nc.vector.tensor_scalar_mul(vrow_sb[:], pvrow[:], 1.0 / n)
                    pbc = psO.tile([P, P], dt.float32, tag="o")
                    nc.tensor.matmul(pbc[:], ones_bc[:], vrow_sb[:])
                    nc.vector.tensor_copy(vbar_bc[:], pbc[:])
                else:
                    pbar = psO.tile([1, 1024], dt.float32, tag="o")
                    if vmerge:
                        for j in range(nt):
                            nc.tensor.matmul(
                                pbar[:, 0:130], ones_t[:], vhat[:, j, :],
                                start=(j == 0), stop=(j == nt - 1),
                            )
                    else:
                        for j in range(nt):
                            nc.tensor.matmul(
                                pbar[:, 0:65], ones_t[:], vh0[:, j, :],
                                start=(j == 0), stop=(j == nt - 1),
                            )
                            nc.tensor.matmul(
                                pbar[:, 512:577], ones_t[:], vh1[:, j, :],
                                start=(j == 0), stop=(j == nt - 1),
                            )
                    vbar_sb = vbpool.tile([1, 130], dt.bfloat16, tag="vbar_sb")
                    if vmerge:
                        nc.vector.tensor_copy(vbar_sb[0:1, 0:130], pbar[:, 0:130])
                    else:
                        nc.vector.tensor_copy(vbar_sb[0:1, 0:65], pbar[:, 0:65])
                        nc.vector.tensor_copy(vbar_sb[0:1, 65:130], pbar[:, 512:577])
                    pbc = psO.tile([P, 130], dt.float32, tag="o")
                    nc.tensor.matmul(pbc[:], ones_bc[:], vbar_sb[:])
                    nc.vector.tensor_copy(vbar_bc[:, 0:64], pbc[:, 0:64])
                    nc.vector.tensor_copy(vbar_bc[:, 64:128], pbc[:, 65:129])

                # ---- attention ----
                outT0 = opool.tile([96, n], dt.float16, tag="outT")
                outT1 = opool.tile([96, n], dt.float16, tag="outT")
                nc.gpsimd.memset(outT0[:], 0.0)
                nc.gpsimd.memset(outT1[:], 0.0)
                for iq in range(niq):
                    emit_probe()
                    i0 = iq * iqw
                    pso = psO.tile([65, 1024], dt.float32, tag="o")
                    e_tiles = {}

                    def emit_out_mm(j):
                        e_p = e_tiles.pop(j)
                        nc.tensor.matmul(
                            pso[:, 0:512], vh0[:, j, :], e_p[:, 0:512],
                            start=(j == 0), stop=(j == nt - 1), skip_group_check=True,
                        )
                        nc.tensor.matmul(
                            pso[:, 512:1024], vh1[:, j, :], e_p[:, 512:1024],
                            start=(j == 0), stop=(j == nt - 1), skip_group_check=True,
                        )

                    for j in range(nt):
                        pss0 = psA.tile([P, 512], dt.float32, tag="ps", name="pss0")
                        pss1 = psA.tile([P, 512], dt.float32, tag="ps", name="pss1")
                        nc.tensor.matmul(
                            pss0[:],
                            kT[0:64, j * P : (j + 1) * P],
                            qT[0:64, i0 : i0 + iqw],
                            tile_position=(0, 0),
                        )
                        nc.tensor.matmul(
                            pss1[:],
                            kT[64:128, j * P : (j + 1) * P],
                            qT[64:128, i0 : i0 + iqw],
                            tile_position=(64, 0),
                        )
                        e_t = epool.tile([P, 1024], dt.bfloat16, tag="e")
                        e_tiles[j] = e_t
                        for h, pss in ((0, pss0), (1, pss1)):
                            esl = e_t[:, h * 512 : (h + 1) * 512]
                            if (2 * j + h) % 3 == 2:
                                nc.vector.tensor_scalar(
                                    esl.bitcast(dt.int16), pss[:],
                                    SCHRAUD_A, gbias2_t[b][:][:, j : j + 1],
                                    AluOpType.mult, AluOpType.add,
                                )
                            else:
                                nc.scalar.activation(
                                    esl, pss[:], AF.Exp, bias=gbias_t[b][:][:, j : j + 1]
                                )
                        # skewed software pipeline: the out-MM for j-skew
                        # issues after j's S-MMs so exp has a full S-pair of
                        # headroom before the in-order PE reaches its consumer
                        if j >= skew:
                            emit_out_mm(j - skew)
                    for jj in range(nt - skew, nt):
                        emit_out_mm(jj)
                    nc.vector.tensor_copy(outT0[0:65, i0 : i0 + iqw], pso[:, 0:512])
                    nc.vector.tensor_copy(outT1[0:65, i0 : i0 + iqw], pso[:, 512:1024])

                # ---- transpose to natural layout, normalize, blend, store ----
                for h, (outT, vsl) in enumerate(((outT0, slice(0, 64)), (outT1, slice(64, 128)))):
                    nat = npool.tile([P, nt, 96], dt.float16, tag="nat")
                    nc.sync.dma_start_transpose(nat[:], outT[:])
                    rcp = fpool.tile([P, nt], dt.float32, tag="rcp")
                    nc.vector.reciprocal(rcp[:], nat[:, :, 64])
                    rcpb = fpool.tile([P, nt], dt.float16, tag="rcpb")
                    nc.vector.tensor_copy(rcpb[:], rcp[:])
                    norm = fpool.tile([P, nt, 65], dt.float16, tag="norm")
                    nc.vector.tensor_tensor(
                        norm[:, :, 0:64], nat[:, :, 0:64],
                        rcpb[:].unsqueeze(2).broadcast_to([P, nt, 64]),
                        AluOpType.mult,
                    )
                    fin = fpool.tile([P, nt, 65], dt.float32, tag="fin")
                    nc.vector.select(
                        fin[:, :, 0:64],
                        mask_t[b][:].unsqueeze(2).broadcast_to([P, nt, 64]),

                        norm[:, :, 0:64],
                        vbar_bc[:, vsl].unsqueeze(1).broadcast_to([P, nt, 64]),
                    )
                    nc.sync.dma_start(
                        out_d[b].rearrange("(t p) c -> p t c", p=P)[:, :, h * 64 : (h + 1) * 64],
                        fin[:, :, 0:64],
                    )
            if probe:
                nc.sync.dma_start(probe_d[:], pr_sc[0:1, 0:4])
                prps_cm.__exit__(None, None, None)
    nc.compile()
    return nc


def _get_graph(b_count=B, n=N, reps=1):
    key = (b_count, n, reps)
    if key not in _GRAPH_CACHE:
        _GRAPH_CACHE[key] = _build_graph(b_count, n, reps)
    return _GRAPH_CACHE[key]


def _prepare_in_maps(Q_vec, K_vec, V_vec, W_Q, W_K, W_V, mask):
    b, n, _ = Q_vec.shape
    scaling = 1.0 / np.sqrt(D)

    def blkdiag(w):
        out = np.zeros((P, P), dtype=BF16)
        wt = w.T.astype(BF16)
        out[0:D, 0:D] = wt
        out[D:P, D:P] = wt
        return out

    wq_np = blkdiag(W_Q.astype(np.float32) * scaling)
    wk_np = blkdiag(W_K)
    wv_np = blkdiag(W_V)

    nt = n // P
    maskf = mask.astype(np.uint8)            # [b, n]
    mask_t = np.ascontiguousarray(maskf.reshape(b, nt, P).transpose(0, 2, 1))  # [b,P,nt]
    gb = (MASK_BIAS * (1.0 - mask.astype(np.float32))).reshape(b, nt, P)
    gbias = np.ascontiguousarray(gb.transpose(0, 2, 1))  # [b, P, nt]
    gbias2 = (gbias * SCHRAUD_A + SCHRAUD_B).astype(np.float32)

    in_maps = []
    for c in range(NCORES):
        sl = slice(c * P, (c + 1) * P)
        m = {
            "xqT": np.ascontiguousarray(
                Q_vec[:, :, sl].astype(BF16).transpose(0, 2, 1)),
            "xkT": np.ascontiguousarray(
                K_vec[:, :, sl].astype(BF16).transpose(0, 2, 1)),
            "xvT": np.ascontiguousarray(
                V_vec[:, :, sl].astype(BF16).transpose(0, 2, 1)),
            "wq": wq_np,
            "wk": wk_np,
            "wv": wv_np,
            "maskt": mask_t,
            "gbias": gbias,
            "gbias2": gbias2,
            "ident": np.eye(P, dtype=BF16),
        }
        in_maps.append(m)
    return in_maps


def run_on_hw(inputs, trace=False, trace_kwargs=None):
    from concourse.bass_utils import run_bass_kernel_spmd

    Q_vec = np.asarray(inputs["Q_vec"], dtype=np.float32)
    K_vec = np.asarray(inputs["K_vec"], dtype=np.float32)
    V_vec = np.asarray(inputs["V_vec"], dtype=np.float32)
    b, n, hd = Q_vec.shape
    nc = _get_graph(b, n)
    in_maps = _prepare_in_maps(
        Q_vec, K_vec, V_vec,
        np.asarray(inputs["W_Q"], dtype=np.float32),
        np.asarray(inputs["W_K"], dtype=np.float32),
        np.asarray(inputs["W_V"], dtype=np.float32),
        np.asarray(inputs["mask"]),
    )
    kw = {}
    if trace:
        kw["trace"] = True
        if trace_kwargs:
            kw["trace_kwargs"] = trace_kwargs
    res = run_bass_kernel_spmd(nc, in_maps, core_ids=list(range(NCORES)), **kw)
    out = np.empty((b, n, hd), dtype=np.float32)
    for c in range(NCORES):
        out[:, :, c * P : (c + 1) * P] = res.results[c]["out"]
    return out, res


def kernel(**inputs):
    out, _ = run_on_hw(inputs, trace=False)
    return out
